# revision 5
# baseline (speedup 1.0000x reference)
"""Cepstrum -> impulse response (Oppenheim recursion) on 8 Trainium2 cores.

Math: the reference recursion h[0]=exp(c[0]); h[n]=(1/n)*sum_m m*c[m]*h[n-m]
is exactly the power-series exponential h = exp-series(c).  Since
H(z) = exp(C(z)) is entire in z^-1, h[n] decays super-exponentially, so a
size-K DFT evaluation  h = IDFT_K(exp(DFT_K(c)))[:512]  is exact to fp32
(aliasing ~ |h[K]| ~ e^-30).  This turns a serial 511-step recurrence into
three dense matmuls + pointwise exp/sin/cos, which map onto the TensorEngine
and ScalarEngine.

Sharding: pure data parallel, batch 65536 -> 8 x 8192 rows.
"""

import math

import numpy as np

import concourse.bass as bass
import concourse.mybir as mybir
import concourse.tile as tile
from concourse.bass_utils import run_bass_kernel_spmd
from concourse.masks import make_identity

F32 = mybir.dt.float32
AF = mybir.ActivationFunctionType

B_TOTAL = 65536
M1 = 100           # cepstral coeffs (order 99 + c0)
N_OUT = 512        # impulse response length
NCORES = 8
ROWS = B_TOTAL // NCORES    # 8192 rows per core

K_DFT = 766        # DFT size; bins 0..383 (rfft), 384*2 spectrum rows
NBINS = 384
NQ = 3             # 128-row chunks per Re/Im half of spectrum
BLK = 512          # batch rows per block (fwd matmul free dim)
NBLK = ROWS // BLK          # 16
TPB = BLK // 128            # batch tiles per block = 4
GROUP = 8          # blocks per ACT-table phase (exp vs trig batching)

def _split_multi_waits(nc):
    """walrus in this container rejects >1 sync-wait on a single instruction
    (setupSyncWait: 'Too many sync wait commands').  Move all but the last
    wait of every instruction onto preceding same-engine NoOps — the engine
    stalls at the NoOps first, which is semantically identical."""
    ctr = 0
    for f in nc.m.functions:
        for bb in f.blocks:
            out = []
            for ins in bb.instructions:
                si = ins.sync_info
                if si is not None and si.on_wait and len(si.on_wait) > 1:
                    waits = list(si.on_wait)
                    for w in waits[:-1]:
                        nop = mybir.InstNoOp(name=f"wsplit-{ctr}", ins=[], outs=[])
                        ctr += 1
                        nop.engine = ins.engine
                        nop.sync_info = mybir.SyncInfo(on_wait=[w], on_update=[])
                        out.append(nop)
                    si.on_wait = [waits[-1]]
                out.append(ins)
            if len(out) != len(bb.instructions):
                bb.instructions[:] = out
    return ctr


def _build_nc():
    nc = bass.Bass()
    c_in = nc.dram_tensor("c", [ROWS, M1], F32, kind="ExternalInput")
    fre = nc.dram_tensor("fre", [M1, NBINS], F32, kind="ExternalInput")
    fim = nc.dram_tensor("fim", [M1, NBINS], F32, kind="ExternalInput")
    gcd = nc.dram_tensor("gc", [128, NQ, N_OUT], F32, kind="ExternalInput")
    gsd = nc.dram_tensor("gs", [128, NQ, N_OUT], F32, kind="ExternalInput")
    h_out = nc.dram_tensor("h", [ROWS, N_OUT], F32, kind="ExternalOutput")

    with tile.TileContext(nc) as tc:
        with (
            tc.tile_pool(name="const", bufs=1) as constp,
            tc.tile_pool(name="cin", bufs=3) as cinp,
            tc.tile_pool(name="ct", bufs=GROUP + 2) as ctp,
            tc.tile_pool(name="esb", bufs=GROUP + 2) as esbp,
            tc.tile_pool(name="hsb", bufs=2) as hsbp,
            tc.tile_pool(name="trig", bufs=2) as trigp,
            tc.tile_pool(name="osb", bufs=4) as osbp,
            tc.tile_pool(name="tp_ps", bufs=2, space="PSUM") as tpps,
            tc.tile_pool(name="fwd_ps", bufs=2, space="PSUM") as fwdps,
            tc.tile_pool(name="out_ps", bufs=3, space="PSUM") as outps,
        ):
            ident = constp.tile([128, 128], F32)
            make_identity(nc, ident)
            fre_sb = constp.tile([M1, NBINS], F32)
            nc.sync.dma_start(out=fre_sb, in_=fre[:, :])
            fim_sb = constp.tile([M1, NBINS], F32)
            nc.sync.dma_start(out=fim_sb, in_=fim[:, :])
            gc_sb = constp.tile([128, NQ, N_OUT], F32)
            nc.sync.dma_start(out=gc_sb, in_=gcd[:, :, :])
            gs_sb = constp.tile([128, NQ, N_OUT], F32)
            nc.sync.dma_start(out=gs_sb, in_=gsd[:, :, :])
            halfpi = constp.tile([128, 1], F32)
            nc.vector.memset(halfpi, math.pi / 2)

            for g0 in range(0, NBLK, GROUP):
                blocks = list(range(g0, min(g0 + GROUP, NBLK)))
                cts = {}
                es = {}
                # Phase A (exp table set): load c, transpose, Re-DFT, exp
                for b in blocks:
                    ctile = cinp.tile([128, TPB, M1], F32, tag="ctile")
                    src = c_in[b * BLK : (b + 1) * BLK, :].rearrange(
                        "(t p) m -> p t m", p=128
                    )
                    nc.sync.dma_start(out=ctile, in_=src)
                    ct = ctp.tile([M1, BLK], F32, tag="ct")
                    for t in range(TPB):
                        ps_t = tpps.tile([128, 128], F32, tag="tp")
                        nc.tensor.transpose(ps_t[:M1, :], ctile[:, t, :], ident)
                        nc.vector.tensor_copy(
                            ct[:, t * 128 : (t + 1) * 128], ps_t[:M1, :]
                        )
                    e_t = esbp.tile([128, NQ, BLK], F32, tag="e")
                    for q in range(NQ):
                        ps_f = fwdps.tile([128, BLK], F32, tag="fwd")
                        nc.tensor.matmul(
                            ps_f,
                            lhsT=fre_sb[:, q * 128 : (q + 1) * 128],
                            rhs=ct,
                            start=True,
                            stop=True,
                        )
                        nc.scalar.activation(out=e_t[:, q, :], in_=ps_f, func=AF.Exp)
                    cts[b] = ct
                    es[b] = e_t
                # Phase B (trig table set) + inverse DFT per block
                for b in blocks:
                    ct = cts[b]
                    e_t = es[b]
                    hre = hsbp.tile([128, NQ, BLK], F32, tag="hre")
                    him = hsbp.tile([128, NQ, BLK], F32, tag="him")
                    for q in range(NQ):
                        ps_f = fwdps.tile([128, BLK], F32, tag="fwd")
                        nc.tensor.matmul(
                            ps_f,
                            lhsT=fim_sb[:, q * 128 : (q + 1) * 128],
                            rhs=ct,
                            start=True,
                            stop=True,
                        )
                        sin_t = trigp.tile([128, BLK], F32, tag="sin")
                        cos_t = trigp.tile([128, BLK], F32, tag="cos")
                        nc.scalar.activation(out=sin_t, in_=ps_f, func=AF.Sin)
                        # cos(x) = sin(x + pi/2); |x| < 1.7 keeps arg within
                        # the ACT Sin accurate range (-pi, pi)
                        nc.scalar.activation(
                            out=cos_t, in_=ps_f, func=AF.Sin, bias=halfpi
                        )
                        nc.vector.tensor_mul(hre[:, q, :], e_t[:, q, :], cos_t)
                        nc.vector.tensor_mul(him[:, q, :], e_t[:, q, :], sin_t)
                    for t in range(TPB):
                        ps_o = outps.tile([128, N_OUT], F32, tag="out")
                        for q in range(NQ):
                            nc.tensor.matmul(
                                ps_o,
                                lhsT=hre[:, q, t * 128 : (t + 1) * 128],
                                rhs=gc_sb[:, q, :],
                                start=(q == 0),
                                stop=False,
                            )
                        for q in range(NQ):
                            nc.tensor.matmul(
                                ps_o,
                                lhsT=him[:, q, t * 128 : (t + 1) * 128],
                                rhs=gs_sb[:, q, :],
                                start=False,
                                stop=(q == NQ - 1),
                            )
                        ob = osbp.tile([128, N_OUT], F32, tag="ob")
                        if t % 2 == 0:
                            nc.vector.tensor_copy(ob, ps_o)
                        else:
                            nc.scalar.copy(ob, ps_o)
                        r0 = b * BLK + t * 128
                        nc.sync.dma_start(out=h_out[r0 : r0 + 128, :], in_=ob)
    _split_multi_waits(nc)
    return nc


_nc_cache = None
_consts_cache = None


def _get_nc():
    global _nc_cache
    if _nc_cache is None:
        _nc_cache = _build_nc()
    return _nc_cache


def _get_consts():
    global _consts_cache
    if _consts_cache is None:
        m = np.arange(M1, dtype=np.float64)
        k = np.arange(NBINS, dtype=np.float64)
        n = np.arange(N_OUT, dtype=np.float64)
        ang_f = 2.0 * np.pi * np.outer(m, k) / K_DFT
        FRE = np.cos(ang_f).astype(np.float32)
        FIM = (-np.sin(ang_f)).astype(np.float32)
        w = np.full(NBINS, 2.0 / K_DFT)
        w[0] = 1.0 / K_DFT
        w[-1] = 1.0 / K_DFT  # Nyquist bin (K even, k = K/2)
        ang_g = 2.0 * np.pi * np.outer(k, n) / K_DFT
        GC = (w[:, None] * np.cos(ang_g)).astype(np.float32)
        GS = (-w[:, None] * np.sin(ang_g)).astype(np.float32)
        gc = np.ascontiguousarray(GC.reshape(NQ, 128, N_OUT).transpose(1, 0, 2))
        gs = np.ascontiguousarray(GS.reshape(NQ, 128, N_OUT).transpose(1, 0, 2))
        _consts_cache = (FRE, FIM, gc, gs)
    return _consts_cache


def _run(c, **spmd_kwargs):
    c = np.ascontiguousarray(np.asarray(c, dtype=np.float32))
    assert c.shape == (B_TOTAL, M1), c.shape
    nc = _get_nc()
    FRE, FIM, gc, gs = _get_consts()
    in_maps = []
    for i in range(NCORES):
        shard = np.ascontiguousarray(c[i * ROWS : (i + 1) * ROWS])
        in_maps.append({"c": shard, "fre": FRE, "fim": FIM, "gc": gc, "gs": gs})
    res = run_bass_kernel_spmd(nc, in_maps, core_ids=list(range(NCORES)), **spmd_kwargs)
    out = np.concatenate([r["h"] for r in res.results], axis=0)
    return out, res


def kernel(c):
    out, _ = _run(c)
    return out


# revision 6
# speedup vs baseline: 3.3943x; 3.3943x over previous
"""Cepstrum -> impulse response (Oppenheim recursion) on 8 Trainium2 cores.

Math: the reference recursion h[0]=exp(c[0]); h[n]=(1/n)*sum_m m*c[m]*h[n-m]
is exactly the power-series exponential h = exp-series(c).  Since
H(z) = exp(C(z)) is entire in z^-1, h[n] decays super-exponentially
(|h[512]| ~ 5e-10), so a K=512 DFT evaluation
    h = IDFT_512(exp(rDFT_512(c)))
is exact to fp32.  This turns the serial 511-step recurrence into three
dense matmuls + pointwise exp/sin/cos on TensorE/ScalarE.

Spectrum packing (K=512, bins 0..256): the 257 Re rows + 255 nonzero Im
rows (Im of bins 0 and 256 are identically 0 for real input) pack into
exactly 512 rows = 4 PE contraction chunks:
  chunk0 = Hre bins   0..127      chunk1 = Hre bins 128..255
  chunk2 = [Nyquist row; Him bins 1..127]
  chunk3 = Him bins 128..255
The Him chunk2 product is computed full-width (lane 0 = E*sin(0) = 0) and
lane 0 is then overwritten with E_nyq = exp(Cre(pi)) via a 1-partition
copy; the IDFT matrix rows are permuted to match.

Sharding: pure data parallel, batch 65536 -> 8 x 8192 rows.
"""

import math
import os

import numpy as np

import concourse.bass as bass
import concourse.mybir as mybir
import concourse.tile as tile
from concourse.bass_utils import run_bass_kernel_spmd
from concourse.masks import make_identity

F32 = mybir.dt.float32
F32R = mybir.dt.float32r
AF = mybir.ActivationFunctionType

B_TOTAL = 65536
M1 = 100           # cepstral coeffs (order 99 + c0)
N_OUT = 512        # impulse response length
NCORES = 8
ROWS = B_TOTAL // NCORES    # 8192 rows per core

K_DFT = 512
NQ = 4             # packed spectrum chunks
BLK = 512          # batch rows per block (fwd matmul free dim)
NBLK = ROWS // BLK          # 16
TPB = BLK // 128            # batch tiles per block = 4
GROUP = 8          # blocks per ACT-table phase (exp vs trig batching)


def _split_multi_waits(nc):
    """walrus in this container rejects >1 sync-wait on a single instruction
    (setupSyncWait: 'Too many sync wait commands').  Move all but the last
    wait of every instruction onto preceding same-engine NoOps — the engine
    stalls at the NoOps first, which is semantically identical."""
    ctr = 0
    for f in nc.m.functions:
        for bb in f.blocks:
            out = []
            for ins in bb.instructions:
                si = ins.sync_info
                if si is not None and si.on_wait and len(si.on_wait) > 1:
                    waits = list(si.on_wait)
                    for w in waits[:-1]:
                        nop = mybir.InstNoOp(name=f"wsplit-{ctr}", ins=[], outs=[])
                        ctr += 1
                        nop.engine = ins.engine
                        nop.sync_info = mybir.SyncInfo(on_wait=[w], on_update=[])
                        out.append(nop)
                    si.on_wait = [waits[-1]]
                out.append(ins)
            if len(out) != len(bb.instructions):
                bb.instructions[:] = out
    return ctr


def _build_nc(use_f32r: bool):
    mmdt = F32R if use_f32r else F32
    nc = bass.Bass()
    c_in = nc.dram_tensor("c", [ROWS, M1], F32, kind="ExternalInput")
    fmat = nc.dram_tensor("fmat", [M1, 513], F32, kind="ExternalInput")
    gmat = nc.dram_tensor("gmat", [128, NQ, N_OUT], F32, kind="ExternalInput")
    h_out = nc.dram_tensor("h", [ROWS, N_OUT], F32, kind="ExternalOutput")

    with tile.TileContext(nc) as tc:
        with (
            tc.tile_pool(name="const", bufs=1) as constp,
            tc.tile_pool(name="cin", bufs=3) as cinp,
            tc.tile_pool(name="ct", bufs=GROUP + 2) as ctp,
            tc.tile_pool(name="esb", bufs=GROUP + 2) as esbp,
            tc.tile_pool(name="hsb", bufs=2) as hsbp,
            tc.tile_pool(name="trig", bufs=2) as trigp,
            tc.tile_pool(name="osb", bufs=4) as osbp,
            tc.tile_pool(name="tp_ps", bufs=2, space="PSUM") as tpps,
            tc.tile_pool(name="fwd_ps", bufs=3, space="PSUM") as fwdps,
            tc.tile_pool(name="out_ps", bufs=3, space="PSUM") as outps,
        ):
            ident = constp.tile([128, 128], F32)
            make_identity(nc, ident)
            f_raw = constp.tile([M1, 513], F32)
            nc.sync.dma_start(out=f_raw, in_=fmat[:, :])
            g_raw = constp.tile([128, NQ, N_OUT], F32)
            nc.sync.dma_start(out=g_raw, in_=gmat[:, :, :])
            if use_f32r:
                f_sb = constp.tile([M1, 513], F32R)
                nc.vector.tensor_copy(f_sb, f_raw)
                g_sb = constp.tile([128, NQ, N_OUT], F32R)
                nc.vector.tensor_copy(g_sb, g_raw)
            else:
                f_sb = f_raw
                g_sb = g_raw
            halfpi = constp.tile([128, 1], F32)
            nc.vector.memset(halfpi, math.pi / 2)

            # F column blocks: [Re0 | Re1 | nyq | Im0 | Im1]
            FQ = [(0, 128), (128, 128), (256, 1), (257, 128), (385, 128)]

            for g0 in range(0, NBLK, GROUP):
                blocks = list(range(g0, min(g0 + GROUP, NBLK)))
                cts = {}
                es = {}
                e2s = {}
                # Phase A (exp table set): load c, transpose, Re-DFT, exp
                for b in blocks:
                    ctile = cinp.tile([128, TPB, M1], F32, tag="ctile")
                    src = c_in[b * BLK : (b + 1) * BLK, :].rearrange(
                        "(t p) m -> p t m", p=128
                    )
                    nc.sync.dma_start(out=ctile, in_=src)
                    ct = ctp.tile([M1, BLK], mmdt, tag="ct")
                    for t in range(TPB):
                        ps_t = tpps.tile([128, 128], F32, tag="tp")
                        nc.tensor.transpose(ps_t[:M1, :], ctile[:, t, :], ident)
                        nc.vector.tensor_copy(
                            ct[:, t * 128 : (t + 1) * 128], ps_t[:M1, :]
                        )
                    e_t = esbp.tile([128, 2, BLK], F32, tag="e")
                    e2_t = esbp.tile([1, BLK], F32, tag="e2")
                    for qi in range(3):
                        o, w = FQ[qi]
                        ps_f = fwdps.tile([128, BLK], F32, tag="fwd")
                        nc.tensor.matmul(
                            ps_f[:w, :],
                            lhsT=f_sb[:, o : o + w],
                            rhs=ct,
                            start=True,
                            stop=True,
                        )
                        dst = e_t[:, qi, :] if qi < 2 else e2_t[:, :]
                        nc.scalar.activation(out=dst, in_=ps_f[:w, :], func=AF.Exp)
                    cts[b] = ct
                    es[b] = e_t
                    e2s[b] = e2_t
                # Phase B (trig table set) + inverse DFT per block
                for b in blocks:
                    ct = cts[b]
                    e_t = es[b]
                    e2_t = e2s[b]
                    spec = hsbp.tile([128, NQ, BLK], mmdt, tag="spec")
                    for qi in range(2):
                        o, w = FQ[3 + qi]
                        ps_f = fwdps.tile([128, BLK], F32, tag="fwd")
                        nc.tensor.matmul(
                            ps_f,
                            lhsT=f_sb[:, o : o + w],
                            rhs=ct,
                            start=True,
                            stop=True,
                        )
                        sin_t = trigp.tile([128, BLK], F32, tag="sin")
                        cos_t = trigp.tile([128, BLK], F32, tag="cos")
                        nc.scalar.activation(out=sin_t, in_=ps_f, func=AF.Sin)
                        # cos(x) = sin(x + pi/2); |x| < 1.7 keeps the arg
                        # within ACT Sin's accurate range (-pi, pi)
                        nc.scalar.activation(
                            out=cos_t, in_=ps_f, func=AF.Sin, bias=halfpi
                        )
                        nc.vector.tensor_mul(spec[:, qi, :], e_t[:, qi, :], cos_t)
                        nc.vector.tensor_mul(spec[:, 2 + qi, :], e_t[:, qi, :], sin_t)
                    # lane 0 of chunk2 (= E0*sin(0) = 0) becomes the Nyquist row
                    nc.vector.tensor_copy(spec[0:1, 2, :], e2_t[:, :])
                    for t in range(TPB):
                        ps_o = outps.tile([128, N_OUT], F32, tag="out")
                        for q in range(NQ):
                            nc.tensor.matmul(
                                ps_o,
                                lhsT=spec[:, q, t * 128 : (t + 1) * 128],
                                rhs=g_sb[:, q, :],
                                start=(q == 0),
                                stop=(q == NQ - 1),
                            )
                        ob = osbp.tile([128, N_OUT], F32, tag="ob")
                        if t % 2 == 0:
                            nc.vector.tensor_copy(ob, ps_o)
                        else:
                            nc.scalar.copy(ob, ps_o)
                        r0 = b * BLK + t * 128
                        nc.sync.dma_start(out=h_out[r0 : r0 + 128, :], in_=ob)
    _split_multi_waits(nc)
    return nc


_nc_cache = {}
_consts_cache = None


def _use_f32r():
    return os.environ.get("KERNEL_F32R", "0") == "1"


def _get_nc():
    key = _use_f32r()
    if key not in _nc_cache:
        _nc_cache[key] = _build_nc(key)
    return _nc_cache[key]


def _get_consts():
    global _consts_cache
    if _consts_cache is None:
        K = float(K_DFT)
        m = np.arange(M1, dtype=np.float64)
        n = np.arange(N_OUT, dtype=np.float64)
        p = np.arange(128, dtype=np.float64)
        F = np.zeros((M1, 513))
        kk = np.arange(257, dtype=np.float64)
        F[:, 0:257] = np.cos(2 * np.pi * np.outer(m, kk) / K)
        F[:, 257:385] = -np.sin(2 * np.pi * np.outer(m, np.arange(128.0)) / K)
        F[:, 385:513] = -np.sin(2 * np.pi * np.outer(m, np.arange(128.0, 256.0)) / K)
        G = np.zeros((128, NQ, N_OUT))
        G[:, 0, :] = (2.0 / K) * np.cos(2 * np.pi * np.outer(p, n) / K)
        G[0, 0, :] *= 0.5  # bin 0 weight 1/K
        G[:, 1, :] = (2.0 / K) * np.cos(2 * np.pi * np.outer(p + 128, n) / K)
        G[:, 2, :] = -(2.0 / K) * np.sin(2 * np.pi * np.outer(p, n) / K)
        G[0, 2, :] = (1.0 / K) * np.cos(np.pi * n)  # Nyquist row: (1/K)(-1)^n
        G[:, 3, :] = -(2.0 / K) * np.sin(2 * np.pi * np.outer(p + 128, n) / K)
        _consts_cache = (
            np.ascontiguousarray(F.astype(np.float32)),
            np.ascontiguousarray(G.astype(np.float32)),
        )
    return _consts_cache


def _run(c, **spmd_kwargs):
    c = np.ascontiguousarray(np.asarray(c, dtype=np.float32))
    assert c.shape == (B_TOTAL, M1), c.shape
    nc = _get_nc()
    F, G = _get_consts()
    in_maps = []
    for i in range(NCORES):
        shard = np.ascontiguousarray(c[i * ROWS : (i + 1) * ROWS])
        in_maps.append({"c": shard, "fmat": F, "gmat": G})
    res = run_bass_kernel_spmd(nc, in_maps, core_ids=list(range(NCORES)), **spmd_kwargs)
    out = np.concatenate([r["h"] for r in res.results], axis=0)
    return out, res


def kernel(c):
    out, _ = _run(c)
    return out
